# revision 12
# baseline (speedup 1.0000x reference)
"""FlyLoRA layer kernel for Trainium2 (8 NeuronCores, data-parallel over tokens).

Computes, for x [4, 4096, 4096], A [32, 4096], B [4096, 32], d [32], k=8:
    y = x @ A.T                      # [B, S, 32]
    mask = top-8 mask of |y + d|     # over the 32 experts
    out = (y * mask) @ B.T * 2.0     # [B, S, 4096]

Sharding: tokens (B*S = 16384) split into 8 contiguous slabs of 2048, one per
core. A/B/d are tiny and replicated. x is pre-tiled on the host so every x
load is 128 partitions x contiguous lines (cheap HWDGE descriptors).

Precision: mm1 (the contraction that decides top-k selection) runs in exact
fp32 on the PE; mm2 and the output store run in bf16 (~3e-3 rel err on output
values, an order of magnitude under the 2e-2 gate) which halves both the mm2
PE time and the store traffic (32 -> 16 MiB per core).

Per core the 2048 tokens run as 5 software-pipelined slices tapered
[256, 512, 512, 512, 128, 128]: slice s-1's top-k chain, mm2 and stores are emitted
between slice s's load+mm1 groups so every engine fills the DMA-wait gaps, and
the final fully-exposed slice is tiny. Loads are issued on the sync HWDGE
queue and stores on the scalar HWDGE queue so a store waiting on compute can
never head-of-line-block a prefetch load. mm1 runs even/odd feature chunks as
two concurrent PE column groups (merged by one vector add; 3-way column
tiling with fp32 operands races on the quadrant-3 XBUS); mm2 replicates
act^T/B^T across 3 partition groups so its K=32 matmuls run row-tiled.
"""

import os

import numpy as np
import ml_dtypes

import concourse.bacc as bacc
import concourse.tile as tile
from concourse import mybir
from concourse.bass_utils import run_bass_kernel_spmd
from concourse.masks import make_identity

F32 = mybir.dt.float32
BF16 = mybir.dt.bfloat16
ALU = mybir.AluOpType
ABS = mybir.ActivationFunctionType.Abs

N_CORES = 8
TOKENS = 16384
TPC = 2048          # tokens per core
D = 4096            # feature dim
R = 32              # experts / lora rank
KC = D // 128       # 32 feature chunks of 128
KC4 = KC // 4       # 8 x-loads per slice (4 feature chunks each)
TSS = [256, 512, 512, 512, 128, 128]   # tapered slice sizes (sum = TPC)
SLICES = len(TSS)
TOK0 = [sum(TSS[:i]) for i in range(SLICES)]
XOFF = [0]          # flat-x element offset of each slice's block
for _ts in TSS[:-1]:
    XOFF.append(XOFF[-1] + D * _ts)

_nc_cache = {}

# exposed for test.py: last BassKernelResults (for exec_time_ns when tracing)
LAST_RESULT = None


def _build_kernel():
    nc = bacc.Bacc(
        "TRN2",
        target_bir_lowering=False,
        debug=False,
        num_devices=N_CORES,
    )
    xT_d = nc.dram_tensor("xT", [TPC * D], F32, kind="ExternalInput").ap()
    atp_d = nc.dram_tensor("ATp", [128, KC * R], F32, kind="ExternalInput").ap()
    bt2_d = nc.dram_tensor("BT2rep", [96, D], BF16, kind="ExternalInput").ap()
    pt_d = nc.dram_tensor("PTrep", [R, 96], BF16, kind="ExternalInput").ap()
    dcol_d = nc.dram_tensor("dcol", [R, 1], F32, kind="ExternalInput").ap()
    out_d = nc.dram_tensor("out", [TPC, D], BF16, kind="ExternalOutput").ap()

    with tile.TileContext(nc) as tc:
        _kernel_body(tc, out_d, xT_d, atp_d, bt2_d, pt_d, dcol_d)
    nc.compile()
    return nc


def _kernel_body(tc, out_d, xT_d, atp_d, bt2_d, pt_d, dcol_d):
    nc = tc.nc

    from contextlib import ExitStack

    with ExitStack() as ctx:
        const = ctx.enter_context(tc.tile_pool(name="const", bufs=1))
        work = ctx.enter_context(tc.tile_pool(name="work", bufs=2))
        blk = ctx.enter_context(tc.tile_pool(name="blk", bufs=2))
        xpool = ctx.enter_context(tc.tile_pool(name="xT", bufs=10))
        ypool = ctx.enter_context(tc.tile_pool(name="ypsum", bufs=2, space="PSUM"))
        tpool = ctx.enter_context(tc.tile_pool(name="tp", bufs=2, space="PSUM"))
        opool = ctx.enter_context(tc.tile_pool(name="opsum", bufs=2, space="PSUM"))
        osb = ctx.enter_context(tc.tile_pool(name="osb", bufs=3))

        # --- constants (on the store queue, which is idle at start, so the
        # x stream owns the load queue from the first instruction) ---
        atp_sb = const.tile([128, KC * R], F32)   # [p, kc*32+r] = A[r, 128*kc+p]
        nc.scalar.dma_start(out=atp_sb[:], in_=atp_d[:])
        bt2_sb = const.tile([96, D], BF16)        # 2*B^T replicated x3 over partitions
        nc.scalar.dma_start(out=bt2_sb[:], in_=bt2_d[:])
        pt_sb = const.tile([R, 96], BF16)         # PT[r, m] = (m % 32 == r)
        nc.scalar.dma_start(out=pt_sb[:], in_=pt_d[:])
        dcol_sb = const.tile([R, 1], F32)         # per-partition bias for |y+d|
        nc.scalar.dma_start(out=dcol_sb[:], in_=dcol_d[:])
        ident = const.tile([128, 128], F32)
        make_identity(nc, ident[:])

        st = [dict() for _ in range(SLICES)]  # per-slice live tiles

        def emit_load_mm1(s, k4):
            # stream one x chunk (4 feature rows x TS tokens); mm1 (fp32)
            # accumulates even/odd feature chunks as two concurrent PE
            # column groups
            ts = TSS[s]
            if k4 == 0:
                st[s]["ypsum"] = ypool.tile([R * 2, ts], F32, tag="yps",
                                            name="yps")
            ypsum = st[s]["ypsum"]
            xt = xpool.tile([128, 4, ts], F32, name="xt")
            off = XOFF[s] + k4 * 512 * ts
            nc.sync.dma_start(
                out=xt[:],
                in_=xT_d[off:off + 512 * ts].rearrange("(p f) -> p f", p=128),
            )
            for c in range(4):
                kc = 4 * k4 + c
                g = kc % 2
                nc.tensor.matmul(
                    ypsum[R * g:R * (g + 1), :],
                    atp_sb[:, R * kc:R * (kc + 1)],
                    xt[:, c, :],
                    start=(kc == g),
                    stop=(kc == KC - 2 + g),
                    tile_position=(0, R * g),
                )

        def emit_chain(s, i):
            # piece i of the per-slice serial chain (top-8 mask of |y+d|)
            t = st[s]
            ts = TSS[s]
            sch = ts // 128
            if i == 0:
                # merge column groups; z^T = |y^T + d|
                ypsum = t["ypsum"]
                t["yg1"] = work.tile([R, ts], F32, name="yg1")
                nc.scalar.copy(t["yg1"][:], ypsum[R:2 * R, :])
                t["yT"] = work.tile([R, ts], F32, name="yT")
                nc.vector.tensor_add(t["yT"][:], ypsum[0:R, :], t["yg1"][:])
                t["zT"] = work.tile([R, ts], F32, name="zT")
                nc.scalar.activation(t["zT"][:], t["yT"][:], ABS,
                                     bias=dcol_sb[:])
            elif i == 1:
                # transpose z^T -> token-major; top-8 of first half
                t["ztok"] = tpool.tile([128, sch * R], F32, tag="tp",
                                       name="ztp")
                for c in range(sch):
                    nc.tensor.transpose(
                        t["ztok"][:, R * c:R * (c + 1)],
                        t["zT"][:, 128 * c:128 * (c + 1)],
                        ident[0:R, 0:R],
                    )
                t["zap"] = work.tile([128, sch * R], F32, name="zap")
                for c in range((sch + 1) // 2):
                    m8 = blk.tile([128, 8], F32, tag="m8", name="m8")
                    zc = t["ztok"][:, R * c:R * (c + 1)]
                    nc.vector.max(out=m8[:], in_=zc)
                    nc.vector.match_replace(
                        out=t["zap"][:, R * c:R * (c + 1)],
                        in_to_replace=m8[:], in_values=zc, imm_value=-1.0,
                    )
            elif i == 2:
                # top-8 of second half; mask = (zap < 0)
                for c in range((sch + 1) // 2, sch):
                    m8 = blk.tile([128, 8], F32, tag="m8", name="m8")
                    zc = t["ztok"][:, R * c:R * (c + 1)]
                    nc.vector.max(out=m8[:], in_=zc)
                    nc.vector.match_replace(
                        out=t["zap"][:, R * c:R * (c + 1)],
                        in_to_replace=m8[:], in_values=zc, imm_value=-1.0,
                    )
                t["mask"] = work.tile([128, sch * R], F32, name="mask")
                nc.vector.tensor_scalar(t["mask"][:], t["zap"][:], 0.0, None,
                                        op0=ALU.is_lt)
            else:
                # transpose mask back; act^T = y^T * mask^T; replicate x3
                maskT = tpool.tile([R, ts], F32, tag="tp", name="mtp")
                for c in range(sch):
                    nc.tensor.transpose(
                        maskT[:, 128 * c:128 * (c + 1)],
                        t["mask"][:, R * c:R * (c + 1)],
                        ident[:],
                    )
                actT = work.tile([R, ts], BF16, name="actT")
                nc.vector.tensor_mul(actT[:], t["yT"][:], maskT[:])
                rep_ps = tpool.tile([96, ts], F32, tag="tp", name="rep")
                nc.tensor.matmul(rep_ps[:], pt_sb[:], actT[:],
                                 start=True, stop=True)
                t["actT4"] = work.tile([96, ts], BF16, name="actT4")
                nc.scalar.copy(t["actT4"][:], rep_ps[:])

        def emit_mm2(s, c):
            # one 128-token chunk: mm2 (bf16, 3-way row-tiled), psum->bf16
            # copies, store on the scalar HWDGE queue
            actT4 = st[s]["actT4"]
            row0 = TOK0[s] + 128 * c
            ot = osb.tile([128, D], BF16, name="ot")
            for h in range(4):
                ps = opool.tile([128, 1024], F32, name="ops")
                for n2 in range(2):
                    j = 2 * h + n2
                    rg = R * (j % 3)
                    nc.tensor.matmul(
                        ps[:, 512 * n2:512 * (n2 + 1)],
                        actT4[rg:rg + R, 128 * c:128 * (c + 1)],
                        bt2_sb[rg:rg + R, 512 * j:512 * (j + 1)],
                        start=True,
                        stop=True,
                    )
                if h % 2 == 0:
                    nc.scalar.copy(ot[:, 1024 * h:1024 * (h + 1)], ps[:])
                else:
                    nc.vector.tensor_copy(ot[:, 1024 * h:1024 * (h + 1)],
                                          ps[:])
            nc.scalar.dma_start(out=out_d[row0:row0 + 128, :], in_=ot[:])

        # software-pipelined emission: slice s-1's chain/mm2/stores are
        # interleaved between slice s's load+mm1 groups so the PE (and the
        # store stream) fill the DMA-wait gaps of the x prefetch
        for s in range(SLICES):
            for k4 in range(KC4):
                if s > 0:
                    if k4 < 4:
                        emit_chain(s - 1, k4)
                    elif k4 - 4 < TSS[s - 1] // 128:
                        emit_mm2(s - 1, k4 - 4)
                emit_load_mm1(s, k4)
        last = SLICES - 1
        for i in range(4):
            emit_chain(last, i)
        for c in range(TSS[last] // 128):
            emit_mm2(last, c)


def _get_nc():
    if "nc" not in _nc_cache:
        _nc_cache["nc"] = _build_kernel()
    return _nc_cache["nc"]


def kernel(x, A, B, d, k):
    global LAST_RESULT
    assert int(k) == 8, f"kernel hardcodes k=8, got {k}"
    x = np.asarray(x, dtype=np.float32)
    A = np.asarray(A, dtype=np.float32)
    B = np.asarray(B, dtype=np.float32)
    d = np.asarray(d, dtype=np.float32)
    assert x.shape == (4, 4096, 4096) and A.shape == (R, D) and B.shape == (D, R)

    X = x.reshape(TOKENS, D)
    xT = X.T                                                          # [D, TOKENS] view
    ATp = np.ascontiguousarray(
        A.T.reshape(KC, 128, R).transpose(1, 0, 2).reshape(128, KC * R)
    )
    BT2 = (np.ascontiguousarray(B.T) * np.float32(2.0)).astype(
        ml_dtypes.bfloat16)                                           # [R, D]
    BT2rep = np.ascontiguousarray(np.tile(BT2, (3, 1)))               # [96, D]
    PTrep = np.zeros((R, 96), dtype=ml_dtypes.bfloat16)
    for g in range(3):
        PTrep[np.arange(R), R * g + np.arange(R)] = 1
    dcol = np.ascontiguousarray(d.reshape(R, 1))

    nc = _get_nc()
    in_maps = []
    for cc in range(N_CORES):
        # flat layout: per slice s, 8 chunks of [128, 4*TS_s]; chunk element
        # [p, 4c+t] = xT[512*k4 + 128*c + p, core_tok0 + TOK0[s] + t]
        parts = []
        for s in range(SLICES):
            ts = TSS[s]
            blk = xT[:, cc * TPC + TOK0[s]: cc * TPC + TOK0[s] + ts]  # [D, ts]
            parts.append(np.ascontiguousarray(
                blk.reshape(KC4, 4, 128, ts).transpose(0, 2, 1, 3)
            ).reshape(-1))
        xflat = np.concatenate(parts)
        assert xflat.size == TPC * D
        in_maps.append({
            "xT": xflat,
            "ATp": ATp,
            "BT2rep": BT2rep,
            "PTrep": PTrep,
            "dcol": dcol,
        })
    trace = bool(int(os.environ.get("KERNEL_TRACE", "0")))
    res = run_bass_kernel_spmd(nc, in_maps, list(range(N_CORES)), trace=trace)
    LAST_RESULT = res
    outs = [np.asarray(res.results[c]["out"]).astype(np.float32)
            for c in range(N_CORES)]
    full = np.concatenate(outs, axis=0)                               # [16384, 4096]
    return full.reshape(4, 4096, 4096)


# revision 13
# speedup vs baseline: 1.0614x; 1.0614x over previous
"""FlyLoRA layer kernel for Trainium2 (8 NeuronCores, data-parallel over tokens).

Computes, for x [4, 4096, 4096], A [32, 4096], B [4096, 32], d [32], k=8:
    y = x @ A.T                      # [B, S, 32]
    mask = top-8 mask of |y + d|     # over the 32 experts
    out = (y * mask) @ B.T * 2.0     # [B, S, 4096]

Sharding: tokens (B*S = 16384) split into 8 contiguous slabs of 2048, one per
core. A/B/d are tiny and replicated. x is pre-tiled on the host so every x
load is 128 partitions x contiguous lines (cheap HWDGE descriptors).

Precision: mm1 (the contraction that decides top-k selection) runs in exact
fp32 on the PE; mm2 and the output store run in bf16 (~3e-3 rel err on output
values, an order of magnitude under the 2e-2 gate) which halves both the mm2
PE time and the store traffic (32 -> 16 MiB per core).

Per core the 2048 tokens run as 5 software-pipelined slices tapered
[512, 512, 512, 256, 128, 128]: slice s-1's top-k chain, mm2 and stores are emitted
between slice s's load+mm1 groups so every engine fills the DMA-wait gaps, and
the final fully-exposed slice is tiny. Loads alternate between the sync HWDGE
and gpsimd SWDGE queues (2/3 of SDMA round-robin bandwidth vs the store
queue, matching the 2:1 load:store byte ratio) and stores go on the scalar
HWDGE queue so a store waiting on compute can
never head-of-line-block a prefetch load. mm1 runs even/odd feature chunks as
two concurrent PE column groups (merged by one vector add; 3-way column
tiling with fp32 operands races on the quadrant-3 XBUS); mm2 replicates
act^T/B^T across 3 partition groups so its K=32 matmuls run row-tiled.
"""

import os

import numpy as np
import ml_dtypes

import concourse.bacc as bacc
import concourse.tile as tile
from concourse import mybir
from concourse.bass_utils import run_bass_kernel_spmd
from concourse.masks import make_identity

F32 = mybir.dt.float32
BF16 = mybir.dt.bfloat16
ALU = mybir.AluOpType
ABS = mybir.ActivationFunctionType.Abs

N_CORES = 8
TOKENS = 16384
TPC = 2048          # tokens per core
D = 4096            # feature dim
R = 32              # experts / lora rank
KC = D // 128       # 32 feature chunks of 128
KC4 = KC // 4       # 8 x-loads per slice (4 feature chunks each)
TSS = [512, 512, 512, 256, 128, 128]   # tapered slice sizes (sum = TPC)
SLICES = len(TSS)
TOK0 = [sum(TSS[:i]) for i in range(SLICES)]
XOFF = [0]          # flat-x element offset of each slice's block
for _ts in TSS[:-1]:
    XOFF.append(XOFF[-1] + D * _ts)

_nc_cache = {}

# exposed for test.py: last BassKernelResults (for exec_time_ns when tracing)
LAST_RESULT = None


def _build_kernel():
    nc = bacc.Bacc(
        "TRN2",
        target_bir_lowering=False,
        debug=False,
        num_devices=N_CORES,
    )
    xT_d = nc.dram_tensor("xT", [TPC * D], F32, kind="ExternalInput").ap()
    atp_d = nc.dram_tensor("ATp", [128, KC * R], F32, kind="ExternalInput").ap()
    bt2_d = nc.dram_tensor("BT2rep", [96, D], BF16, kind="ExternalInput").ap()
    pt_d = nc.dram_tensor("PTrep", [R, 96], BF16, kind="ExternalInput").ap()
    dcol_d = nc.dram_tensor("dcol", [R, 1], F32, kind="ExternalInput").ap()
    out_d = nc.dram_tensor("out", [TPC, D], BF16, kind="ExternalOutput").ap()

    with tile.TileContext(nc) as tc:
        _kernel_body(tc, out_d, xT_d, atp_d, bt2_d, pt_d, dcol_d)
    nc.compile()
    return nc


def _kernel_body(tc, out_d, xT_d, atp_d, bt2_d, pt_d, dcol_d):
    nc = tc.nc

    from contextlib import ExitStack

    with ExitStack() as ctx:
        const = ctx.enter_context(tc.tile_pool(name="const", bufs=1))
        work = ctx.enter_context(tc.tile_pool(name="work", bufs=2))
        blk = ctx.enter_context(tc.tile_pool(name="blk", bufs=2))
        xpool = ctx.enter_context(tc.tile_pool(name="xT", bufs=10))
        ypool = ctx.enter_context(tc.tile_pool(name="ypsum", bufs=2, space="PSUM"))
        tpool = ctx.enter_context(tc.tile_pool(name="tp", bufs=2, space="PSUM"))
        opool = ctx.enter_context(tc.tile_pool(name="opsum", bufs=2, space="PSUM"))
        osb = ctx.enter_context(tc.tile_pool(name="osb", bufs=3))

        # --- constants (on the store queue, which is idle at start, so the
        # x stream owns the load queue from the first instruction) ---
        atp_sb = const.tile([128, KC * R], F32)   # [p, kc*32+r] = A[r, 128*kc+p]
        nc.scalar.dma_start(out=atp_sb[:], in_=atp_d[:])
        bt2_sb = const.tile([96, D], BF16)        # 2*B^T replicated x3 over partitions
        nc.scalar.dma_start(out=bt2_sb[:], in_=bt2_d[:])
        pt_sb = const.tile([R, 96], BF16)         # PT[r, m] = (m % 32 == r)
        nc.scalar.dma_start(out=pt_sb[:], in_=pt_d[:])
        dcol_sb = const.tile([R, 1], F32)         # per-partition bias for |y+d|
        nc.scalar.dma_start(out=dcol_sb[:], in_=dcol_d[:])
        ident = const.tile([128, 128], F32)
        make_identity(nc, ident[:])

        st = [dict() for _ in range(SLICES)]  # per-slice live tiles

        def emit_load_mm1(s, k4):
            # stream one x chunk (4 feature rows x TS tokens); mm1 (fp32)
            # accumulates even/odd feature chunks as two concurrent PE
            # column groups
            ts = TSS[s]
            if k4 == 0:
                st[s]["ypsum"] = ypool.tile([R * 2, ts], F32, tag="yps",
                                            name="yps")
            ypsum = st[s]["ypsum"]
            xt = xpool.tile([128, 4, ts], F32, name="xt")
            off = XOFF[s] + k4 * 512 * ts
            ldq = nc.sync if k4 % 2 == 0 else nc.gpsimd
            ldq.dma_start(
                out=xt[:],
                in_=xT_d[off:off + 512 * ts].rearrange("(p f) -> p f", p=128),
            )
            for c in range(4):
                kc = 4 * k4 + c
                g = kc % 2
                nc.tensor.matmul(
                    ypsum[R * g:R * (g + 1), :],
                    atp_sb[:, R * kc:R * (kc + 1)],
                    xt[:, c, :],
                    start=(kc == g),
                    stop=(kc == KC - 2 + g),
                    tile_position=(0, R * g),
                )

        def emit_chain(s, i):
            # piece i of the per-slice serial chain (top-8 mask of |y+d|)
            t = st[s]
            ts = TSS[s]
            sch = ts // 128
            if i == 0:
                # merge column groups; z^T = |y^T + d|
                ypsum = t["ypsum"]
                t["yg1"] = work.tile([R, ts], F32, name="yg1")
                nc.scalar.copy(t["yg1"][:], ypsum[R:2 * R, :])
                t["yT"] = work.tile([R, ts], F32, name="yT")
                nc.vector.tensor_add(t["yT"][:], ypsum[0:R, :], t["yg1"][:])
                t["zT"] = work.tile([R, ts], F32, name="zT")
                nc.scalar.activation(t["zT"][:], t["yT"][:], ABS,
                                     bias=dcol_sb[:])
            elif i == 1:
                # transpose z^T -> token-major; top-8 of first half
                t["ztok"] = tpool.tile([128, sch * R], F32, tag="tp",
                                       name="ztp")
                for c in range(sch):
                    nc.tensor.transpose(
                        t["ztok"][:, R * c:R * (c + 1)],
                        t["zT"][:, 128 * c:128 * (c + 1)],
                        ident[0:R, 0:R],
                    )
                t["zap"] = work.tile([128, sch * R], F32, name="zap")
                for c in range((sch + 1) // 2):
                    m8 = blk.tile([128, 8], F32, tag="m8", name="m8")
                    zc = t["ztok"][:, R * c:R * (c + 1)]
                    nc.vector.max(out=m8[:], in_=zc)
                    nc.vector.match_replace(
                        out=t["zap"][:, R * c:R * (c + 1)],
                        in_to_replace=m8[:], in_values=zc, imm_value=-1.0,
                    )
            elif i == 2:
                # top-8 of second half; mask = (zap < 0)
                for c in range((sch + 1) // 2, sch):
                    m8 = blk.tile([128, 8], F32, tag="m8", name="m8")
                    zc = t["ztok"][:, R * c:R * (c + 1)]
                    nc.vector.max(out=m8[:], in_=zc)
                    nc.vector.match_replace(
                        out=t["zap"][:, R * c:R * (c + 1)],
                        in_to_replace=m8[:], in_values=zc, imm_value=-1.0,
                    )
                t["mask"] = work.tile([128, sch * R], F32, name="mask")
                nc.vector.tensor_scalar(t["mask"][:], t["zap"][:], 0.0, None,
                                        op0=ALU.is_lt)
            else:
                # transpose mask back; act^T = y^T * mask^T; replicate x3
                maskT = tpool.tile([R, ts], F32, tag="tp", name="mtp")
                for c in range(sch):
                    nc.tensor.transpose(
                        maskT[:, 128 * c:128 * (c + 1)],
                        t["mask"][:, R * c:R * (c + 1)],
                        ident[:],
                    )
                actT = work.tile([R, ts], BF16, name="actT")
                nc.vector.tensor_mul(actT[:], t["yT"][:], maskT[:])
                rep_ps = tpool.tile([96, ts], F32, tag="tp", name="rep")
                nc.tensor.matmul(rep_ps[:], pt_sb[:], actT[:],
                                 start=True, stop=True)
                t["actT4"] = work.tile([96, ts], BF16, name="actT4")
                nc.scalar.copy(t["actT4"][:], rep_ps[:])

        def emit_mm2(s, c):
            # one 128-token chunk: mm2 (bf16, 3-way row-tiled), psum->bf16
            # copies, store on the scalar HWDGE queue
            actT4 = st[s]["actT4"]
            row0 = TOK0[s] + 128 * c
            ot = osb.tile([128, D], BF16, name="ot")
            for h in range(4):
                ps = opool.tile([128, 1024], F32, name="ops")
                for n2 in range(2):
                    j = 2 * h + n2
                    rg = R * (j % 3)
                    nc.tensor.matmul(
                        ps[:, 512 * n2:512 * (n2 + 1)],
                        actT4[rg:rg + R, 128 * c:128 * (c + 1)],
                        bt2_sb[rg:rg + R, 512 * j:512 * (j + 1)],
                        start=True,
                        stop=True,
                    )
                if h % 2 == 0:
                    nc.scalar.copy(ot[:, 1024 * h:1024 * (h + 1)], ps[:])
                else:
                    nc.vector.tensor_copy(ot[:, 1024 * h:1024 * (h + 1)],
                                          ps[:])
            nc.scalar.dma_start(out=out_d[row0:row0 + 128, :], in_=ot[:])

        # software-pipelined emission: slice s-1's chain/mm2/stores are
        # interleaved between slice s's load+mm1 groups so the PE (and the
        # store stream) fill the DMA-wait gaps of the x prefetch
        for s in range(SLICES):
            for k4 in range(KC4):
                if s > 0:
                    if k4 < 4:
                        emit_chain(s - 1, k4)
                    elif k4 - 4 < TSS[s - 1] // 128:
                        emit_mm2(s - 1, k4 - 4)
                emit_load_mm1(s, k4)
        last = SLICES - 1
        for i in range(4):
            emit_chain(last, i)
        for c in range(TSS[last] // 128):
            emit_mm2(last, c)


def _get_nc():
    if "nc" not in _nc_cache:
        _nc_cache["nc"] = _build_kernel()
    return _nc_cache["nc"]


def kernel(x, A, B, d, k):
    global LAST_RESULT
    assert int(k) == 8, f"kernel hardcodes k=8, got {k}"
    x = np.asarray(x, dtype=np.float32)
    A = np.asarray(A, dtype=np.float32)
    B = np.asarray(B, dtype=np.float32)
    d = np.asarray(d, dtype=np.float32)
    assert x.shape == (4, 4096, 4096) and A.shape == (R, D) and B.shape == (D, R)

    X = x.reshape(TOKENS, D)
    xT = X.T                                                          # [D, TOKENS] view
    ATp = np.ascontiguousarray(
        A.T.reshape(KC, 128, R).transpose(1, 0, 2).reshape(128, KC * R)
    )
    BT2 = (np.ascontiguousarray(B.T) * np.float32(2.0)).astype(
        ml_dtypes.bfloat16)                                           # [R, D]
    BT2rep = np.ascontiguousarray(np.tile(BT2, (3, 1)))               # [96, D]
    PTrep = np.zeros((R, 96), dtype=ml_dtypes.bfloat16)
    for g in range(3):
        PTrep[np.arange(R), R * g + np.arange(R)] = 1
    dcol = np.ascontiguousarray(d.reshape(R, 1))

    nc = _get_nc()
    in_maps = []
    for cc in range(N_CORES):
        # flat layout: per slice s, 8 chunks of [128, 4*TS_s]; chunk element
        # [p, 4c+t] = xT[512*k4 + 128*c + p, core_tok0 + TOK0[s] + t]
        parts = []
        for s in range(SLICES):
            ts = TSS[s]
            blk = xT[:, cc * TPC + TOK0[s]: cc * TPC + TOK0[s] + ts]  # [D, ts]
            parts.append(np.ascontiguousarray(
                blk.reshape(KC4, 4, 128, ts).transpose(0, 2, 1, 3)
            ).reshape(-1))
        xflat = np.concatenate(parts)
        assert xflat.size == TPC * D
        in_maps.append({
            "xT": xflat,
            "ATp": ATp,
            "BT2rep": BT2rep,
            "PTrep": PTrep,
            "dcol": dcol,
        })
    trace = bool(int(os.environ.get("KERNEL_TRACE", "0")))
    res = run_bass_kernel_spmd(nc, in_maps, list(range(N_CORES)), trace=trace)
    LAST_RESULT = res
    outs = [np.asarray(res.results[c]["out"]).astype(np.float32)
            for c in range(N_CORES)]
    full = np.concatenate(outs, axis=0)                               # [16384, 4096]
    return full.reshape(4, 4096, 4096)
